# revision 75
# baseline (speedup 1.0000x reference)
"""CostVolume kernel for Trainium2 (8 NeuronCores, batch-sharded).

out[b,h,w,(di,dj)] = mean_c( prv[b,h,w,c] * nxt_pad[b,h+di,w+dj,c] ),  r=4, d=9.

Measured-HW design notes (per core, 2 batches; ~196 us vs 236 us baseline):
  - Band matmul: per 16x8-pixel patch, M=128 pixels x N=384 (24x16 nxt
    window), contracting C=192 as two uniform K=96 chunks. The PE
    moving-read costs ceil(K/64) "beats" per column and the hardware
    util-throttle (50% duty, ~60% of the run) clamps 1-beat columns to
    2-beat time, so uniform 2-beat K=96 shapes that pipeline back-to-back
    at 320ns/matmul are optimal; mixed-K emission breaks pipelining.
  - Quad PSUM tiles [128, 4, 512] (4 banks, bufs=2): 4 patches matmul'd
    chunk-major, evacuated by two half-partition copies (DVE + ACT).
  - band64 SBUF [128, NJ, 256]: partition half 0-63 keeps window cols
    0:256, half 64-127 keeps 128:384 - contiguous 8KB/partition, so the
    store is ONE full-efficiency DMA per (b,i): 13.1 MB/core instead of
    25.2 (and no sub-512B descriptor penalty, which costs 2x fabric time).
  - Ring discipline (critical): the scalar/vector engines run the
    PSUM->SBUF copies and must stay DMA-free mid-stream - a dma_start
    hitting HWDGE flow control blocks the copies behind it and stalls the
    PE via PSUM recycling. All loads go on sync (prv + just-in-time nxt
    slices), stores on gpsimd SWDGE, memsets hoisted to the top.
  - Host prep: prv scaled by 1/C -> bf16 patch-major [b, c, I, J, 128];
    nxt -> bf16 [b, c, 128, 136] (cols pre-padded; rows memset on device).
  - Host gather: out[p, J, d] = band64[b, I, p, J, idx] with
    idx = (((p>>3)&7) + di)*16 + (p&7) + dj (uniform for both halves).
"""

import numpy as np
import ml_dtypes

B, H, W, C = 16, 128, 128, 192
R = 4
D = 2 * R + 1  # 9
N_CORES = 8
B_LOC = B // N_CORES  # 2
# The PE util-throttle clamps sustained matmuls to 2-beat column time
# regardless of K<=128, so two uniform K=96 chunks are optimal (identical
# shapes pipeline back-to-back at 320ns per 384-col matmul; mixed-K
# emission measures ~20-25% slower from shape-switch pipeline breaks).
C0 = 96
C1 = C - C0  # 96
PH, PW = 16, 8  # patch size (h, w); PH*PW = 128 = M
WH, WW = PH + 2 * R, PW + 2 * R  # 24, 16 window
NB = WH * WW  # 384 band columns per patch
NI = H // PH  # 8 patch rows
NJ = W // PW  # 16 patch cols
HP = H + 2 * R  # 136 padded rows (SBUF)
WP = W + 2 * R  # 136 padded cols (HBM + SBUF)
QW = 256  # band window width per 64-partition half (contiguous in SBUF)
NSL = 4  # h-slices per nxt load

_CACHED = {}


def _build_nc():
    import concourse.mybir as mybir
    from concourse.bacc import Bacc
    from concourse.tile import TileContext

    fp32 = mybir.dt.float32
    bf16 = mybir.dt.bfloat16

    nc = Bacc(
        "TRN2",
        target_bir_lowering=False,
        debug=False,
        num_devices=N_CORES,
    )

    prv_d = nc.dram_tensor(
        "prv_t", [B_LOC, C, NI, NJ, PH * PW], bf16, kind="ExternalInput"
    )
    nxt_d = nc.dram_tensor("nxt_p", [B_LOC, C, H, WP], bf16, kind="ExternalInput")
    band_d = nc.dram_tensor(
        "band", [B_LOC, NI, PH * PW, NJ, QW], bf16, kind="ExternalOutput"
    )

    with TileContext(nc) as tc:
        with (
            tc.tile_pool(name="nxt_pool", bufs=2) as nxt_pool,
            tc.tile_pool(name="prv_pool", bufs=2) as prv_pool,
            tc.tile_pool(name="band_pool", bufs=4) as band_pool,
            tc.tile_pool(name="psum_pool", bufs=2, space="PSUM") as psum_pool,
        ):
            # allocate both batches' nxt tiles up front; memset the row
            # borders first thing on the gpsimd queue so the batch-1 loads
            # never wait behind stores
            nxt_tiles = {}
            for b in range(B_LOC):
                n0 = nxt_pool.tile([C0, HP, WP], bf16, tag="nxt_a")
                n1 = nxt_pool.tile([C1, HP, WP], bf16, tag="nxt_b")
                for n in (n0, n1):
                    nc.gpsimd.memset(n[:, 0:R, :], 0.0)
                    nc.gpsimd.memset(n[:, R + H : HP, :], 0.0)
                nxt_tiles[b] = (n0, n1)

            def load_nxt_slice(b, s, ring1=None):
                # one 32-row slice of both chunks on the sync ring (the
                # scalar/vector engines must stay DMA-free mid-stream: a
                # dma_start hitting HWDGE flow control blocks the copies
                # behind it; ring1 override is safe only at startup)
                n0, n1 = nxt_tiles[b]
                lo, hi = H * s // NSL, H * (s + 1) // NSL
                nc.sync.dma_start(
                    n0[:, R + lo : R + hi, :], nxt_d[b, 0:C0, lo:hi, :]
                )
                (ring1 or nc.sync).dma_start(
                    n1[:, R + lo : R + hi, :], nxt_d[b, C0:C, lo:hi, :]
                )



            pairs = [(b, i) for b in range(B_LOC) for i in range(NI)]
            prv_tiles = {}

            def load_prv(k):
                b, i = pairs[k]
                p0 = prv_pool.tile([C0, NJ, PH * PW], bf16, tag="prv_a")
                p1 = prv_pool.tile([C1, NJ, PH * PW], bf16, tag="prv_b")
                nc.sync.dma_start(p0[:], prv_d[b, 0:C0, i])
                nc.sync.dma_start(p1[:], prv_d[b, C0:C, i])
                prv_tiles[k] = (p0, p1)

            # startup in arrival-need order: p0 then nxt chunk-0 slice 0 on
            # sync (first c1 quads), chunk-1 slice 0 alone on the empty
            # scalar queue (done long before the first copy needs it), p1
            # next on sync (first c2s).
            b0, i0 = pairs[0]
            p0_f = prv_pool.tile([C0, NJ, PH * PW], bf16, tag="prv_a")
            p1_f = prv_pool.tile([C1, NJ, PH * PW], bf16, tag="prv_b")
            nc.sync.dma_start(p0_f[:], prv_d[b0, 0:C0, i0])
            load_nxt_slice(0, 0, ring1=nc.scalar)
            nc.sync.dma_start(p1_f[:], prv_d[b0, C0:C, i0])
            prv_tiles[0] = (p0_f, p1_f)
            load_prv(1)
            load_nxt_slice(0, 1)

            # just-in-time nxt slice schedule: (batch, slice) emitted at
            # the top of iteration k
            nxt_sched = {
                0: (0, 2), 1: (0, 3),
                2: (1, 0), 3: (1, 1), 4: (1, 2), 5: (1, 3),
            }

            # copy-engine rotation: DVE, ACT (Pool cannot access PSUM on TRN2)
            cp_idx = 0

            for k in range(len(pairs)):
                b, i = pairs[k]
                if k + 2 < len(pairs):
                    load_prv(k + 2)
                if k in nxt_sched:
                    load_nxt_slice(*nxt_sched[k])
                p0, p1 = prv_tiles.pop(k)
                n0, n1 = nxt_tiles[b]
                # band64[p, J, c]: partition half 0-63 holds window cols
                # 0:256, half 64-127 holds cols 128:384 - contiguous per
                # partition, so the store is one 8KB/partition DMA.
                band = band_pool.tile([PH * PW, NJ, QW], bf16, tag="band_sb")
                r0 = slice(i * PH, i * PH + WH)
                for tq in range(NJ // 4):
                    # one 4-bank psum tile per quad (4 patches); two
                    # half-partition copies evacuate it (one per engine)
                    ps = psum_pool.tile([PH * PW, 4, 512], fp32, tag="band_ps")
                    for m in range(4):
                        j = 4 * tq + m
                        cj = slice(j * PW, j * PW + WW)
                        nc.tensor.matmul(
                            ps[:, m, 0:NB], p0[:, j, :], n0[:, r0, cj],
                            start=True, stop=False,
                        )
                    for m in range(4):
                        j = 4 * tq + m
                        cj = slice(j * PW, j * PW + WW)
                        nc.tensor.matmul(
                            ps[:, m, 0:NB], p1[:, j, :], n1[:, r0, cj],
                            start=False, stop=True,
                        )
                    ja = slice(4 * tq, 4 * tq + 4)
                    if tq % 2 == 0:
                        nc.vector.tensor_copy(
                            band[0:64, ja, :], ps[0:64, :, 0:QW]
                        )
                        nc.scalar.copy(
                            band[64:128, ja, :], ps[64:128, :, NB - QW : NB]
                        )
                    else:
                        nc.scalar.copy(band[0:64, ja, :], ps[0:64, :, 0:QW])
                        nc.vector.tensor_copy(
                            band[64:128, ja, :], ps[64:128, :, NB - QW : NB]
                        )
                if k >= len(pairs) - 2:
                    # pipeline the final stores: sub-store each quad's
                    # columns as soon as its copies land
                    for tq in range(NJ // 4):
                        ja = slice(4 * tq, 4 * tq + 4)
                        nc.sync.dma_start(band_d[b, i, :, ja], band[:, ja, :])
                else:
                    nc.gpsimd.dma_start(band_d[b, i], band[:])

    nc.finalize()
    return nc


def _get_nc():
    if "nc" not in _CACHED:
        _CACHED["nc"] = _build_nc()
    return _CACHED["nc"]


def _host_prep(prv, nxt):
    """prv: scale by 1/C, bf16, patch-major [b, c, I, J, 128].
    nxt: bf16 [b, c, 128, 136] zero-padded cols only."""
    bf16 = ml_dtypes.bfloat16
    prv_t = (np.asarray(prv, dtype=np.float32) * (1.0 / C)).transpose(0, 3, 1, 2)
    prv_t = prv_t.reshape(B, C, NI, PH, NJ, PW).transpose(0, 1, 2, 4, 3, 5)
    prv_t = np.ascontiguousarray(prv_t.reshape(B, C, NI, NJ, PH * PW)).astype(bf16)
    nxt_t = np.asarray(nxt, dtype=np.float32).transpose(0, 3, 1, 2).astype(bf16)
    nxt_p = np.zeros((B, C, H, WP), dtype=bf16)
    nxt_p[:, :, :, R : R + W] = nxt_t
    return prv_t, nxt_p


def _make_in_maps(prv, nxt):
    prv_t, nxt_p = _host_prep(prv, nxt)
    return [
        {
            "prv_t": prv_t[i * B_LOC : (i + 1) * B_LOC],
            "nxt_p": nxt_p[i * B_LOC : (i + 1) * B_LOC],
        }
        for i in range(N_CORES)
    ]


# gather index over the per-half 256-wide window:
# c[p, di, dj] = (((p>>3)&7) + di)*16 + (p&7) + dj  (uniform for both halves)
_p = np.arange(PH * PW)
_di, _dj = np.meshgrid(np.arange(D), np.arange(D), indexing="ij")
_GIDX = (
    (((_p >> 3) & 7)[:, None, None] + _di[None]) * WW
    + (_p & 7)[:, None, None]
    + _dj[None]
).reshape(1, 1, PH * PW, 1, D * D)  # [1,1,128,1,81]


def _gather_band(band8):
    """band8: [B_LOC, NI, 128, NJ, QW] bf16 -> out [B_LOC, H, W, D*D] f32."""
    arr = np.asarray(band8, dtype=np.float32)  # [b, I, p, J, QW]
    idx = np.broadcast_to(_GIDX, arr.shape[:4] + (D * D,))
    out = np.take_along_axis(arr, idx, axis=-1)  # [b, I, p, J, 81]
    out = out.reshape(B_LOC, NI, PH, PW, NJ, D * D)  # p = (i, j)
    out = out.transpose(0, 1, 2, 4, 3, 5)  # [b, I, i, J, j, 81]
    return np.ascontiguousarray(out.reshape(B_LOC, H, W, D * D))


def kernel(prv, nxt, search_range):
    from concourse.bass_utils import run_bass_kernel_spmd

    assert int(search_range) == R
    prv = np.asarray(prv)
    nxt = np.asarray(nxt)
    assert prv.shape == (B, H, W, C), prv.shape

    in_maps = _make_in_maps(prv, nxt)

    nc = _get_nc()
    res = run_bass_kernel_spmd(nc, in_maps, list(range(N_CORES)))

    out = np.empty((B, H, W, D * D), dtype=np.float32)
    for i in range(N_CORES):
        out[i * B_LOC : (i + 1) * B_LOC] = _gather_band(res.results[i]["band"])
    return out
